# revision 12
# baseline (speedup 1.0000x reference)
"""Bass/Trainium2 kernel for batched masked-Kabsch RMSD (nn_Coords2RMSD).

Strategy (per NeuronCore, SPMD across 8 cores):
  - Host sorts the 4096 rows by num_atoms into 64 global groups of 64 rows
    (8 lanes x 8 cores). Per group, atom capacity is rounded to 128-atom
    chunks; rows are zero-padded to the group cap (masking happens on host).
  - Host packs, per core, an atom-major bf16 tensor z[128, TC]: for each
    (group q, chunk k) a 49-column block [x lanes b=0..7 (3 comps each) |
    y lanes | ones]. The PE engine computes the Gram matrix Z^T Z per group,
    accumulated over chunks in PSUM: one 49x49 Gram holds the 3x3
    cross-covariance C per lane, Gxx/Gyy second moments, and Sx/Sy sums
    (via the ones column) -- all atom reductions ride the matmul stream.
  - Extraction: PSUM -> SBUF copy (ScalarE), then PE transposes rearrange
    the per-group Grams into a [64 groups, 2401] stats tile; strided
    broadcast APs address each quantity per lane.
  - Final stage in two 32-group passes (pass 0 overlaps the remaining
    matmul stream): centered C, K = C^T C eigenvalues via the trigonometric
    method with DVE polynomial atan/cos (no activation-table switches),
    Kabsch det sign, RMSD. Only the Sqrt table is used (preloaded early).
"""

import numpy as np
import ml_dtypes

import concourse.bass as bass
import concourse.mybir as mybir
from concourse.tile import TileContext, ScopedClock
from concourse.masks import make_identity

F32 = mybir.dt.float32
BF16 = mybir.dt.bfloat16
OP = mybir.AluOpType
AF = mybir.ActivationFunctionType

N_CORES = 8
GROUPS = 64           # global groups == stats partition dim
LANES = 8             # rows per group per core
GROUP_ROWS = LANES * N_CORES  # 64 sorted rows per group
CHUNK = 128           # atoms per matmul chunk (contraction partitions)
ZCOLS = 6 * LANES + 1  # 49: x(24) | y(24) | ones
NBLK = 8              # matmul blocks of 8 groups (one PSUM bank each)
BLK = GROUPS // NBLK  # 8 groups per block
PP = ZCOLS * ZCOLS    # 2401 stats cols per group
HALF = GROUPS // 2    # final math runs in two 32-group passes


def _poly_coeffs(f, lo, hi, deg):
    u = np.linspace(lo, hi, 2048)
    c = np.polynomial.chebyshev.Chebyshev.fit(u, f(u), deg)
    return list(c.convert(kind=np.polynomial.Polynomial).coef)

# atan(t)/t as a polynomial in u = t^2, u in [0, 1]
ATAN_C = _poly_coeffs(
    lambda u: np.arctan(np.sqrt(np.maximum(u, 1e-12))) / np.sqrt(np.maximum(u, 1e-12)),
    1e-9, 1.0, 6,
)
# cos(x) as a polynomial in u = x^2, u in [0, (pi/3)^2]
COS_C = _poly_coeffs(
    lambda u: np.cos(np.sqrt(np.maximum(u, 0.0))), 0.0, (np.pi / 3) ** 2, 4
)


# ---------------------------------------------------------------------------
# TileContext tail patch: this walrus build accepts at most ONE sync-wait
# command per instruction and no sem-eq waits, so the stock drain + EVSEM
# butterfly fails codegen. Emit a ge-wait-only tail instead.
# ---------------------------------------------------------------------------
def _patched_drain_and_barrier(self, tick_clock, wait_clock):
    nc = self.nc
    dummy = nc.gpsimd.nop()
    wait_clock.add_sem_waits(dummy.ins, ScopedClock({None: tick_clock.global_clock}))
    waits = list(dummy.ins.sync_info.on_wait) if dummy.ins.sync_info else []
    if dummy.ins.sync_info:
        dummy.ins.sync_info = mybir.SyncInfo(on_wait=[], on_update=[])

    bsem = nc.alloc_semaphore(f"tail_bsem_{nc.next_id()}")
    dsem = nc.alloc_semaphore(f"tail_dsem_{nc.next_id()}")
    n_eng = 0
    for eng in nc.engines.values():
        eng.drain()
        eng.sem_inc(bsem, 1)
        n_eng += 1
    nc.gpsimd.wait_ge(bsem, n_eng)
    for w in waits:
        n = nc.gpsimd.nop()
        n.ins.sync_info = mybir.SyncInfo(on_wait=[w], on_update=[])
    nc.gpsimd.sem_inc(dsem, 1)
    for eng in nc.engines.values():
        if eng is not nc.gpsimd:
            eng.wait_ge(dsem, 1)

    popped = nc._tile_sem_poison_stack.pop()
    assert popped is self._sem_poison
    nc.clear_and_free_semaphores(list(self.sems.allocated().values()))
    nc.gpsimd.sem_clear(bsem)
    nc.gpsimd.sem_clear(dsem)


def install_tile_patch():
    TileContext._drain_and_barrier = _patched_drain_and_barrier


# ---------------------------------------------------------------------------
# BIR post-pass: this walrus build accepts at most one sync-wait command per
# instruction (none on Drain). Tile's sem-assigner can attach several, so
# split extras onto same-engine NoOps inserted just before the instruction.
# ---------------------------------------------------------------------------
_orig_to_json_bytes = bass.Bass.to_json_bytes


def _split_multiwait_json(self) -> bytes:
    import json

    raw = _orig_to_json_bytes(self)
    m = json.loads(raw)
    ctr = 0
    changed = False
    for f in m.get("functions", []):
        for blk in f.get("blocks", []):
            insts = blk.get("instructions", [])
            out = []
            for inst in insts:
                si = inst.get("sync_info")
                ow = (si or {}).get("on_wait") or []
                opc = str(inst.get("opcode", inst.get("type", "")))
                limit = 0 if opc == "Drain" else 1
                if len(ow) > limit:
                    keep = ow[len(ow) - limit :] if limit else []
                    moved = ow[: len(ow) - limit] if limit else ow
                    for w in moved:
                        ctr += 1
                        out.append(
                            {
                                "debug": inst.get("debug", 0),
                                "engine": inst["engine"],
                                "ins": [],
                                "name": f"WS-{ctr}-{inst['name']}",
                                "opcode": "NoOp",
                                "outs": [],
                                "sync_info": {"on_update": [], "on_wait": [w]},
                            }
                        )
                    si["on_wait"] = keep
                    changed = True
                out.append(inst)
            blk["instructions"] = out
    if not changed:
        return raw
    return json.dumps(m).encode()


bass.Bass.to_json_bytes = _split_multiwait_json


# ---------------------------------------------------------------------------
# Final math emitter: per 32-group pass, [32, K]-shaped fp32 tiles.
# ---------------------------------------------------------------------------
class _FM:
    def __init__(self, nc, pool, h):
        self.nc = nc
        self.pool = pool
        self.h = h  # pass index (for tile tags)
        self.n = 0

    def t(self, k=LANES):
        self.n += 1
        return self.pool.tile(
            [HALF, k], F32, tag=f"fm{self.h}_{self.n}", name=f"fm{self.h}_{self.n}"
        )

    @staticmethod
    def _w(a):
        return int(np.prod(a.shape[1:]))

    def tt(self, a, b, op):
        o = self.t(self._w(a))
        self.nc.vector.tensor_tensor(o[:], a, b, op)
        return o[:]

    def mul(self, a, b):
        return self.tt(a, b, OP.mult)

    def add(self, a, b):
        return self.tt(a, b, OP.add)

    def sub(self, a, b):
        return self.tt(a, b, OP.subtract)

    def ts(self, a, s, op):
        o = self.t(self._w(a))
        self.nc.vector.tensor_scalar(o[:], a, float(s), None, op)
        return o[:]

    def ts2(self, a, s1, s2, op0, op1):
        o = self.t(self._w(a))
        self.nc.vector.tensor_scalar(o[:], a, float(s1), float(s2), op0, op1)
        return o[:]

    def stt(self, a, s, b, op0, op1):
        """(a op0 s) op1 b"""
        o = self.t(self._w(a))
        self.nc.vector.scalar_tensor_tensor(o[:], a, float(s), b, op0, op1)
        return o[:]

    def sqrt(self, a, k=None):
        o = self.t(k if k is not None else self._w(a))
        self.nc.scalar.activation(o[:], a, AF.Sqrt)
        return o[:]

    def recip(self, a):
        o = self.t(self._w(a))
        self.nc.vector.reciprocal(o[:], a)
        return o[:]

    def poly_u(self, u, coeffs):
        """Evaluate poly(u) (coeffs low->high) via Horner."""
        cs = list(coeffs)
        h = self.ts2(u, cs[-1], cs[-2], OP.mult, OP.add)
        for c in reversed(cs[:-2]):
            hu = self.stt(h, 1.0, u, OP.mult, OP.mult)
            h = self.ts(hu, c, OP.add)
        return h


def _emit_final_pass(nc, pool, h, stats, meta_ap, out_ap):
    """stats: [HALF, PP] AP for this pass. meta/out: [HALF, LANES] APs."""
    fm = _FM(nc, pool, h)
    P = HALF

    def seg(base, width):
        return stats[:, base : base + width]

    # ---- wide quantity APs (order (i, j, b) after permute) ----
    # Rxy(i,j,b): col = 49*(24+3b+j) + 3b+i = 1176 + 150b + 49j + i
    rxy = seg(1176, 1200).rearrange("p (b r) -> p b r", b=LANES)
    rxy = rxy[:, :, 0:147].rearrange("p b (j r2) -> p b j r2", j=3)[:, :, :, 0:3]
    rxy = rxy.rearrange("p b j i -> p i j b")
    # Sx(i,b): col = 2352 + 3b + i ; Sy(j,b): col = 2376 + 3b + j
    sxw = (
        seg(2352, 24)
        .rearrange("p (b i) -> p b i", b=LANES)
        .broadcast_to([P, LANES, 3, 3])
        .rearrange("p b i j -> p i j b")
    )
    syw = (
        seg(2376, 24)
        .rearrange("p (b j) -> p b j", b=LANES)
        .broadcast_to([P, LANES, 3, 3])
        .rearrange("p b j i -> p i j b")
    )

    n_ap = meta_ap
    rn = fm.recip(n_ap)
    rnw = rn.broadcast_to([P, LANES, 3, 3]).rearrange("p b i j -> p i j b")

    # ---- C = Rxy - Sx Sy / n  (wide [32, 72], layout (i, j, b)) ----
    sxsy = fm.t(72)
    nc.vector.tensor_tensor(
        sxsy[:].rearrange("p (i j b) -> p i j b", i=3, j=3), sxw, syw, OP.mult
    )
    corr = fm.t(72)
    nc.vector.tensor_tensor(
        corr[:].rearrange("p (i j b) -> p i j b", i=3, j=3),
        sxsy[:].rearrange("p (i j b) -> p i j b", i=3, j=3),
        rnw,
        OP.mult,
    )
    Ct = fm.t(72)
    nc.vector.tensor_tensor(
        Ct[:].rearrange("p (i j b) -> p i j b", i=3, j=3),
        rxy,
        corr[:].rearrange("p (i j b) -> p i j b", i=3, j=3),
        OP.subtract,
    )

    def C(i, j):
        return Ct[:, (3 * i + j) * LANES : (3 * i + j + 1) * LANES]

    # ---- gx, gy ----
    # Gxx_ii: col = 150b + 50i ; Gyy_ii: col = 1200 + 150b + 50i
    def diag_sum(base):
        a = [
            stats[:, base + 50 * i : base + 50 * i + 150 * (LANES - 1) + 1 : 150]
            for i in range(3)
        ]
        return fm.add(fm.add(a[0], a[1]), a[2])

    gxr = diag_sum(0)
    gyr = diag_sum(1200)

    def Sx(i):
        return stats[:, 2352 + i : 2352 + i + 3 * (LANES - 1) + 1 : 3]

    def Sy(j):
        return stats[:, 2376 + j : 2376 + j + 3 * (LANES - 1) + 1 : 3]

    sx2 = fm.add(fm.add(fm.mul(Sx(0), Sx(0)), fm.mul(Sx(1), Sx(1))), fm.mul(Sx(2), Sx(2)))
    sy2 = fm.add(fm.add(fm.mul(Sy(0), Sy(0)), fm.mul(Sy(1), Sy(1))), fm.mul(Sy(2), Sy(2)))
    gx = fm.sub(gxr, fm.mul(sx2, rn))
    gy = fm.sub(gyr, fm.mul(sy2, rn))

    # ---- K = C^T C via one wide product + fold over i ----
    c4 = Ct[:].rearrange("p (i a b) -> p i a b", i=3, a=3)
    in0 = c4.broadcast_to([P, 3, 3, LANES, 3]).rearrange("p i a b bb -> p a bb i b")
    in1 = c4.broadcast_to([P, 3, 3, LANES, 3]).rearrange("p i bb b a -> p a bb i b")
    P3 = fm.t(216)
    nc.vector.tensor_tensor(
        P3[:].rearrange("p (a bb i b) -> p a bb i b", a=3, bb=3, i=3), in0, in1, OP.mult
    )
    p3v = P3[:].rearrange("p (a bb i b) -> p a bb i b", a=3, bb=3, i=3)
    kk01 = fm.t(72)
    nc.vector.tensor_tensor(
        kk01[:].rearrange("p (a bb b) -> p a bb b", a=3, bb=3),
        p3v[:, :, :, 0, :],
        p3v[:, :, :, 1, :],
        OP.add,
    )
    kkt = fm.t(72)
    nc.vector.tensor_tensor(
        kkt[:].rearrange("p (a bb b) -> p a bb b", a=3, bb=3),
        kk01[:].rearrange("p (a bb b) -> p a bb b", a=3, bb=3),
        p3v[:, :, :, 2, :],
        OP.add,
    )

    def kk(a, b):
        return kkt[:, (3 * a + b) * LANES : (3 * a + b + 1) * LANES]

    # ---- det(C) sign ----
    m0 = fm.sub(fm.mul(C(1, 1), C(2, 2)), fm.mul(C(1, 2), C(2, 1)))
    m1 = fm.sub(fm.mul(C(1, 0), C(2, 2)), fm.mul(C(1, 2), C(2, 0)))
    m2 = fm.sub(fm.mul(C(1, 0), C(2, 1)), fm.mul(C(1, 1), C(2, 0)))
    detC = fm.add(fm.sub(fm.mul(C(0, 0), m0), fm.mul(C(0, 1), m1)), fm.mul(C(0, 2), m2))

    # ---- eigen setup: q, p ----
    q = fm.ts(fm.add(fm.add(kk(0, 0), kk(1, 1)), kk(2, 2)), 1.0 / 3.0, OP.mult)
    kd = [fm.sub(kk(a, a), q) for a in range(3)]
    p2 = fm.add(fm.add(fm.mul(kd[0], kd[0]), fm.mul(kd[1], kd[1])), fm.mul(kd[2], kd[2]))
    xsq = fm.add(
        fm.add(fm.mul(kk(0, 1), kk(0, 1)), fm.mul(kk(0, 2), kk(0, 2))),
        fm.mul(kk(1, 2), kk(1, 2)),
    )
    p2 = fm.stt(xsq, 2.0, p2, OP.mult, OP.add)
    p2c = fm.ts(fm.ts(p2, 1.0 / 6.0, OP.mult), 1e-30, OP.max)
    p = fm.sqrt(p2c)

    # ---- det(K - qI), r clamped ----
    k01, k02, k12 = kk(0, 1), kk(0, 2), kk(1, 2)
    d0 = fm.mul(kd[0], fm.sub(fm.mul(kd[1], kd[2]), fm.mul(k12, k12)))
    d1 = fm.mul(k01, fm.sub(fm.mul(k01, kd[2]), fm.mul(k12, k02)))
    d2 = fm.mul(k02, fm.sub(fm.mul(k01, k12), fm.mul(kd[1], k02)))
    detKq = fm.add(fm.sub(d0, d1), d2)
    rp = fm.recip(p)
    r = fm.mul(fm.mul(fm.ts(detKq, 0.5, OP.mult), rp), fm.mul(rp, rp))
    r = fm.ts(fm.ts(r, 1.0, OP.min), -1.0, OP.max)

    # ---- acos(r)/3 via |r| fold + polynomial atan ----
    rabs = fm.stt(r, -1.0, r, OP.mult, OP.max)
    onemr = fm.ts2(rabs, -1.0, 1.0, OP.mult, OP.add)
    onepr = fm.ts(rabs, 1.0, OP.add)
    u = fm.mul(onemr, fm.recip(onepr))
    su = fm.sqrt(u)  # t = sqrt(u), A = 2 atan(t)
    at = fm.mul(su, fm.poly_u(u, ATAN_C))  # atan(t) = t * P(t^2)
    # acr = 2 at (1 - 2 rneg) + pi rneg ; phi = acr / 3
    rneg = fm.ts(r, 0.0, OP.is_lt)
    sgn = fm.ts2(rneg, -2.0, 1.0, OP.mult, OP.add)
    t1 = fm.mul(at, sgn)
    phi = fm.stt(rneg, float(np.pi / 3.0), fm.ts(t1, 2.0 / 3.0, OP.mult), OP.mult, OP.add)
    # c1 = cos(phi); c3m = cos(pi/3 - phi): pack both args, one poly chain
    w = fm.t(2 * LANES)
    nc.vector.tensor_copy(w[:, 0:LANES], phi)
    nc.vector.tensor_scalar(
        w[:, LANES : 2 * LANES], phi, -1.0, float(np.pi / 3.0), OP.mult, OP.add
    )
    wu = fm.tt(w[:], w[:], OP.mult)  # [32, 16]
    cw = fm.poly_u(wu, COS_C)
    c1 = cw[:, 0:LANES]
    c3m = cw[:, LANES : 2 * LANES]

    # ---- eigenvalues, packed sqrt ----
    p2x = fm.ts(p, 2.0, OP.mult)
    lt = fm.t(3 * LANES)
    nc.vector.tensor_tensor(lt[:, 0:LANES], q, fm.mul(p2x, c1), OP.add)  # l1
    nc.vector.tensor_tensor(
        lt[:, 2 * LANES : 3 * LANES], q, fm.mul(p2x, c3m), OP.subtract
    )  # l3
    nc.vector.tensor_tensor(
        lt[:, LANES : 2 * LANES],
        fm.stt(q, 3.0, lt[:, 0:LANES], OP.mult, OP.subtract),
        lt[:, 2 * LANES : 3 * LANES],
        OP.subtract,
    )  # l2 = 3q - l1 - l3
    ltc = fm.ts(lt[:], 0.0, OP.max)  # [32, 24] clamp
    s = fm.sqrt(ltc, 3 * LANES)

    # ---- trace with Kabsch sign, rmsd ----
    neg = fm.ts(detC, 0.0, OP.is_lt)
    d = fm.ts2(neg, -2.0, 1.0, OP.mult, OP.add)
    tr = fm.add(
        fm.add(s[:, 0:LANES], s[:, LANES : 2 * LANES]),
        fm.mul(d, s[:, 2 * LANES : 3 * LANES]),
    )
    diff = fm.stt(tr, -2.0, fm.add(gx, gy), OP.mult, OP.add)
    msd = fm.mul(diff, rn)
    rmsd = fm.sqrt(fm.ts(msd, 0.0, OP.max))
    nc.vector.tensor_copy(out_ap, rmsd)


# ---------------------------------------------------------------------------
# Program builder. chunks: per-group chunk counts (len 64, same on all cores).
# ---------------------------------------------------------------------------
def build_program(chunks):
    chunks = list(chunks)
    assert len(chunks) == GROUPS
    colstart = np.concatenate([[0], np.cumsum(np.asarray(chunks) * ZCOLS)]).astype(int)
    TC = int(colstart[-1])

    install_tile_patch()
    nc = bass.Bass()
    z_d = nc.dram_tensor("z", [CHUNK, TC], BF16, kind="ExternalInput")
    meta_d = nc.dram_tensor("meta", [GROUPS, LANES], F32, kind="ExternalInput")
    out_d = nc.dram_tensor("out", [GROUPS, LANES], F32, kind="ExternalOutput")

    ZT = 16  # input DMA granularity: 4 groups per tile
    GPT = GROUPS // ZT

    with TileContext(nc) as tc:
        with (
            tc.tile_pool(name="const", bufs=1) as constp,
            tc.tile_pool(name="z", bufs=1) as zp,
            tc.tile_pool(name="pcopy", bufs=1) as pcp,
            tc.tile_pool(name="stats", bufs=1) as statp,
            tc.tile_pool(name="psum1", bufs=2, space="PSUM") as ps1p,
            tc.tile_pool(name="psum2", bufs=1, space="PSUM") as ps2p,
        ):
            # Input tiles first: start the stream as early as possible.
            zt = []
            for t in range(ZT):
                c0 = int(colstart[t * GPT])
                c1 = int(colstart[(t + 1) * GPT])
                tile = zp.tile([CHUNK, c1 - c0], BF16, tag=f"z{t}", name=f"z{t}")
                nc.sync.dma_start(out=tile[:], in_=z_d[:, c0:c1])
                zt.append((tile, c0))

            meta_t = constp.tile([GROUPS, LANES], F32)
            nc.sync.dma_start(out=meta_t[:], in_=meta_d[:])
            ident = constp.tile([ZCOLS, ZCOLS], F32)
            make_identity(nc, ident[:])
            # Pre-load the Sqrt activation table (the only table we use).
            scr = constp.tile([GROUPS, 1], F32)
            nc.vector.memset(scr[:], 1.0)
            nc.scalar.activation(scr[:], scr[:], AF.Sqrt)

            P_all = pcp.tile([ZCOLS, GROUPS * ZCOLS], F32, tag="Pall")
            # psum2: 5 bank-sized tiles, 10 (last 9) Gram columns each.
            ps2 = [
                ps2p.tile(
                    [GROUPS, (10 if t < 4 else 9) * ZCOLS],
                    F32,
                    tag=f"ps2{t}",
                    name=f"ps2{t}",
                )
                for t in range(5)
            ]
            stats = statp.tile([GROUPS, PP], F32)
            out_t = statp.tile([GROUPS, LANES], F32)

            def emit_block(j):
                p1 = ps1p.tile([ZCOLS, BLK * ZCOLS], F32, tag="p1", name=f"p1_{j}")
                for g in range(BLK):
                    q = j * BLK + g
                    tile, c0 = zt[q // GPT]
                    tgt = p1[:, ZCOLS * g : ZCOLS * (g + 1)]
                    nchunks = chunks[q]
                    for k in range(nchunks):
                        o = int(colstart[q]) - c0 + k * ZCOLS
                        zs = tile[:, o : o + ZCOLS]
                        nc.tensor.matmul(
                            tgt, zs, zs, start=(k == 0), stop=(k == nchunks - 1)
                        )
                o = j * BLK * ZCOLS
                nc.scalar.activation(
                    P_all[:, o : o + BLK * ZCOLS], p1[:], AF.Identity
                )

            def emit_extract(h):
                # Transpose groups [32h, 32h+32): per Gram column c, one
                # transpose [49, 32] -> [32, 49] into psum2 partitions 32h+.
                for c in range(ZCOLS):
                    src = P_all[:, h * HALF * ZCOLS + c : (h + 1) * HALF * ZCOLS : ZCOLS]
                    t = c // 10
                    dst = ps2[t][
                        h * HALF : (h + 1) * HALF, ZCOLS * (c - 10 * t) : ZCOLS * (c - 10 * t + 1)
                    ]
                    nc.tensor.transpose(dst, src, ident[:])
                # PSUM -> stats SBUF: split across ScalarE and DVE.
                for t in range(5):
                    w = (10 if t < 4 else 9) * ZCOLS
                    dst = stats[h * HALF : (h + 1) * HALF, 490 * t : 490 * t + w]
                    src = ps2[t][h * HALF : (h + 1) * HALF, :]
                    if t < 3:
                        nc.scalar.activation(dst, src, AF.Identity)
                    else:
                        nc.vector.tensor_copy(dst, src)

            def emit_final(h):
                _emit_final_pass(
                    nc,
                    statp,
                    h,
                    stats[h * HALF : (h + 1) * HALF, :],
                    meta_t[h * HALF : (h + 1) * HALF, :],
                    out_t[h * HALF : (h + 1) * HALF, :],
                )

            # Schedule: blocks 0..3 (pass-0 groups), block 4, then pass-0
            # extraction+math (overlaps blocks 5..7 streaming), blocks 5..7,
            # pass-1 extraction+math.
            for j in range(4):
                emit_block(j)
            emit_block(4)
            emit_extract(0)
            emit_final(0)
            for j in range(5, NBLK):
                emit_block(j)
            emit_extract(1)
            emit_final(1)

            nc.sync.dma_start(out=out_d[:], in_=out_t[:])

    return nc


# ---------------------------------------------------------------------------
# Host side
# ---------------------------------------------------------------------------
def plan_shards(num_atoms):
    na = np.asarray(num_atoms).astype(np.int64)
    B = na.shape[0]
    assert B == GROUPS * GROUP_ROWS, f"unsupported batch {B}"
    order = np.argsort(na, kind="stable")
    caps = na[order].reshape(GROUPS, GROUP_ROWS)[:, -1]
    chunks = np.maximum(1, -(-caps // CHUNK)).astype(int)  # ceil
    return order, chunks


def shard_inputs(coords_input, coords_target, num_atoms, order, chunks):
    B, f = coords_input.shape
    nmax = f // 3
    na = np.asarray(num_atoms).astype(np.int64)
    x3 = coords_input.reshape(B, nmax, 3)
    y3 = coords_target.reshape(B, nmax, 3)
    colstart = np.concatenate([[0], np.cumsum(chunks * ZCOLS)]).astype(int)
    TC = int(colstart[-1])

    in_maps = []
    for c in range(N_CORES):
        z = np.zeros((CHUNK, TC), dtype=ml_dtypes.bfloat16)
        meta = np.empty((GROUPS, LANES), np.float32)
        for v in np.unique(chunks):
            qs = np.where(chunks == v)[0]
            nq = len(qs)
            A = int(v) * CHUNK
            # rows for (q, b): order[q*64 + b*8 + c]
            ridx = order[
                (qs[:, None] * GROUP_ROWS) + np.arange(LANES)[None, :] * N_CORES + c
            ]
            nar = na[ridx]  # [nq, LANES]
            meta[qs, :] = nar.astype(np.float32)
            mask = (np.arange(A)[None, None, :] < nar[:, :, None]).astype(np.float32)
            xa = x3[ridx.ravel(), :A, :].reshape(nq, LANES, A, 3) * mask[..., None]
            ya = y3[ridx.ravel(), :A, :].reshape(nq, LANES, A, 3) * mask[..., None]
            xt = xa.reshape(nq, LANES, int(v), CHUNK, 3).transpose(0, 2, 3, 1, 4)
            yt = ya.reshape(nq, LANES, int(v), CHUNK, 3).transpose(0, 2, 3, 1, 4)
            buf = np.empty((nq, int(v), CHUNK, ZCOLS), np.float32)
            buf[..., 0 : 3 * LANES] = xt.reshape(nq, int(v), CHUNK, 3 * LANES)
            buf[..., 3 * LANES : 6 * LANES] = yt.reshape(nq, int(v), CHUNK, 3 * LANES)
            buf[..., 6 * LANES] = 1.0
            colidx = (
                colstart[qs][:, None] + np.arange(int(v) * ZCOLS)[None, :]
            ).ravel()
            z[:, colidx] = (
                buf.transpose(2, 0, 1, 3).reshape(CHUNK, nq * int(v) * ZCOLS)
            ).astype(ml_dtypes.bfloat16)
        in_maps.append({"z": z, "meta": meta})
    return in_maps


def unshard_outputs(results, order, B):
    out = np.empty(B, dtype=np.float32)
    for c in range(N_CORES):
        o = np.asarray(results[c]["out"], np.float32)  # [GROUPS, LANES]
        q = np.arange(GROUPS)[:, None]
        b = np.arange(LANES)[None, :]
        rows = order[q * GROUP_ROWS + b * N_CORES + c]
        out[rows] = o
    return out


# ---------------------------------------------------------------------------
# Entry point: full inputs in, full output out. Shards across 8 NeuronCores.
# ---------------------------------------------------------------------------
_PROG_CACHE = {}


def _get_program(chunks):
    key = tuple(int(v) for v in chunks)
    if key not in _PROG_CACHE:
        _PROG_CACHE[key] = build_program(list(key))
    return _PROG_CACHE[key]


def kernel(coords_input, coords_target, num_atoms):
    from concourse.bass_utils import run_bass_kernel_spmd

    x = np.ascontiguousarray(np.asarray(coords_input, dtype=np.float32))
    y = np.ascontiguousarray(np.asarray(coords_target, dtype=np.float32))
    na = np.asarray(num_atoms).astype(np.int64)
    B = x.shape[0]

    order, chunks = plan_shards(na)
    in_maps = shard_inputs(x, y, na, order, chunks)
    nc = _get_program(chunks)
    res = run_bass_kernel_spmd(nc, in_maps, core_ids=list(range(N_CORES)))
    return unshard_outputs(res.results, order, B).astype(np.float32)


# revision 16
# speedup vs baseline: 1.1489x; 1.1489x over previous
"""Bass/Trainium2 kernel for batched masked-Kabsch RMSD (nn_Coords2RMSD).

Strategy (per NeuronCore, SPMD across 8 cores):
  - Host sorts the 4096 rows by num_atoms into 64 global groups of 64 rows
    (8 lanes x 8 cores). Per group, atom capacity is rounded to 128-atom
    chunks; rows are zero-padded to the group cap (masking happens on host).
  - Host packs, per core, an atom-major bf16 tensor z[128, TC]: for each
    (group q, chunk k) a 49-column block [x lanes b=0..7 (3 comps each) |
    y lanes | ones]. The PE engine computes the Gram matrix Z^T Z per group,
    accumulated over chunks in PSUM: one 49x49 Gram holds the 3x3
    cross-covariance C per lane, Gxx/Gyy second moments, and Sx/Sy sums
    (via the ones column) -- all atom reductions ride the matmul stream.
  - Extraction: PSUM -> SBUF copy (ScalarE), then PE transposes rearrange
    the per-group Grams into a [64 groups, 2401] stats tile; strided
    broadcast APs address each quantity per lane.
  - Final stage in two 32-group passes (pass 0 overlaps the remaining
    matmul stream): centered C, K = C^T C eigenvalues via the trigonometric
    method with DVE polynomial atan/cos (no activation-table switches),
    Kabsch det sign, RMSD. Only the Sqrt table is used (preloaded early).
"""

import numpy as np
import ml_dtypes

import concourse.bass as bass
import concourse.mybir as mybir
from concourse.tile import TileContext, ScopedClock
from concourse.masks import make_identity

F32 = mybir.dt.float32
BF16 = mybir.dt.bfloat16
OP = mybir.AluOpType
AF = mybir.ActivationFunctionType

N_CORES = 8
GROUPS = 64           # global groups == stats partition dim
LANES = 8             # rows per group per core
GROUP_ROWS = LANES * N_CORES  # 64 sorted rows per group
CHUNK = 128           # atoms per matmul chunk (contraction partitions)
ZCOLS = 6 * LANES + 1  # 49: x(24) | y(24) | ones
NBLK = 8              # matmul blocks of 8 groups (one PSUM bank each)
BLK = GROUPS // NBLK  # 8 groups per block
PP = ZCOLS * ZCOLS    # 2401 stats cols per group
HALF = GROUPS // 2    # final math runs in two 32-group passes


def _poly_coeffs(f, lo, hi, deg):
    u = np.linspace(lo, hi, 2048)
    c = np.polynomial.chebyshev.Chebyshev.fit(u, f(u), deg)
    return list(c.convert(kind=np.polynomial.Polynomial).coef)

# atan(t)/t as a polynomial in u = t^2, u in [0, 1]
ATAN_C = _poly_coeffs(
    lambda u: np.arctan(np.sqrt(np.maximum(u, 1e-12))) / np.sqrt(np.maximum(u, 1e-12)),
    1e-9, 1.0, 4,
)
# cos(x) as a polynomial in u = x^2, u in [0, (pi/3)^2]
COS_C = _poly_coeffs(
    lambda u: np.cos(np.sqrt(np.maximum(u, 0.0))), 0.0, (np.pi / 3) ** 2, 3
)


# ---------------------------------------------------------------------------
# TileContext tail patch: this walrus build accepts at most ONE sync-wait
# command per instruction and no sem-eq waits, so the stock drain + EVSEM
# butterfly fails codegen. Emit a ge-wait-only tail instead.
# ---------------------------------------------------------------------------
def _patched_drain_and_barrier(self, tick_clock, wait_clock):
    nc = self.nc
    dummy = nc.gpsimd.nop()
    wait_clock.add_sem_waits(dummy.ins, ScopedClock({None: tick_clock.global_clock}))
    waits = list(dummy.ins.sync_info.on_wait) if dummy.ins.sync_info else []
    if dummy.ins.sync_info:
        dummy.ins.sync_info = mybir.SyncInfo(on_wait=[], on_update=[])

    bsem = nc.alloc_semaphore(f"tail_bsem_{nc.next_id()}")
    dsem = nc.alloc_semaphore(f"tail_dsem_{nc.next_id()}")
    n_eng = 0
    for eng in nc.engines.values():
        eng.drain()
        eng.sem_inc(bsem, 1)
        n_eng += 1
    nc.gpsimd.wait_ge(bsem, n_eng)
    for w in waits:
        n = nc.gpsimd.nop()
        n.ins.sync_info = mybir.SyncInfo(on_wait=[w], on_update=[])
    nc.gpsimd.sem_inc(dsem, 1)
    for eng in nc.engines.values():
        if eng is not nc.gpsimd:
            eng.wait_ge(dsem, 1)

    popped = nc._tile_sem_poison_stack.pop()
    assert popped is self._sem_poison
    nc.clear_and_free_semaphores(list(self.sems.allocated().values()))
    nc.gpsimd.sem_clear(bsem)
    nc.gpsimd.sem_clear(dsem)


def install_tile_patch():
    TileContext._drain_and_barrier = _patched_drain_and_barrier


# ---------------------------------------------------------------------------
# BIR post-pass: this walrus build accepts at most one sync-wait command per
# instruction (none on Drain). Tile's sem-assigner can attach several, so
# split extras onto same-engine NoOps inserted just before the instruction.
# ---------------------------------------------------------------------------
_orig_to_json_bytes = bass.Bass.to_json_bytes


def _split_multiwait_json(self) -> bytes:
    import json

    raw = _orig_to_json_bytes(self)
    m = json.loads(raw)
    ctr = 0
    changed = False
    for f in m.get("functions", []):
        for blk in f.get("blocks", []):
            insts = blk.get("instructions", [])
            out = []
            for inst in insts:
                si = inst.get("sync_info")
                ow = (si or {}).get("on_wait") or []
                opc = str(inst.get("opcode", inst.get("type", "")))
                limit = 0 if opc == "Drain" else 1
                if len(ow) > limit:
                    keep = ow[len(ow) - limit :] if limit else []
                    moved = ow[: len(ow) - limit] if limit else ow
                    for w in moved:
                        ctr += 1
                        out.append(
                            {
                                "debug": inst.get("debug", 0),
                                "engine": inst["engine"],
                                "ins": [],
                                "name": f"WS-{ctr}-{inst['name']}",
                                "opcode": "NoOp",
                                "outs": [],
                                "sync_info": {"on_update": [], "on_wait": [w]},
                            }
                        )
                    si["on_wait"] = keep
                    changed = True
                out.append(inst)
            blk["instructions"] = out
    if not changed:
        return raw
    return json.dumps(m).encode()


bass.Bass.to_json_bytes = _split_multiwait_json


# ---------------------------------------------------------------------------
# Final math emitter: per 32-group pass, [32, K]-shaped fp32 tiles.
# ---------------------------------------------------------------------------
class _FM:
    def __init__(self, nc, pool, h):
        self.nc = nc
        self.pool = pool
        self.h = h  # pass index (for tile tags)
        self.n = 0

    def t(self, k=LANES):
        self.n += 1
        return self.pool.tile(
            [GROUPS, k], F32, tag=f"fm{self.h}_{self.n}", name=f"fm{self.h}_{self.n}"
        )

    @staticmethod
    def _w(a):
        return int(np.prod(a.shape[1:]))

    def tt(self, a, b, op):
        o = self.t(self._w(a))
        self.nc.vector.tensor_tensor(o[:], a, b, op)
        return o[:]

    def mul(self, a, b):
        return self.tt(a, b, OP.mult)

    def add(self, a, b):
        return self.tt(a, b, OP.add)

    def sub(self, a, b):
        return self.tt(a, b, OP.subtract)

    def ts(self, a, s, op):
        o = self.t(self._w(a))
        self.nc.vector.tensor_scalar(o[:], a, float(s), None, op)
        return o[:]

    def ts2(self, a, s1, s2, op0, op1):
        o = self.t(self._w(a))
        self.nc.vector.tensor_scalar(o[:], a, float(s1), float(s2), op0, op1)
        return o[:]

    def stt(self, a, s, b, op0, op1):
        """(a op0 s) op1 b"""
        o = self.t(self._w(a))
        self.nc.vector.scalar_tensor_tensor(o[:], a, float(s), b, op0, op1)
        return o[:]

    def sqrt(self, a, k=None):
        o = self.t(k if k is not None else self._w(a))
        self.nc.scalar.activation(o[:], a, AF.Sqrt)
        return o[:]

    def recip(self, a):
        o = self.t(self._w(a))
        self.nc.vector.reciprocal(o[:], a)
        return o[:]

    def poly_u(self, u, coeffs):
        """Evaluate poly(u) (coeffs low->high) via Horner."""
        cs = list(coeffs)
        h = self.ts2(u, cs[-1], cs[-2], OP.mult, OP.add)
        for c in reversed(cs[:-2]):
            hu = self.stt(h, 1.0, u, OP.mult, OP.mult)
            h = self.ts(hu, c, OP.add)
        return h


def _emit_final_pass(nc, pool, h, stats, meta_ap, out_ap):
    """Single pass over all GROUPS partitions. stats: [GROUPS, PP] AP."""
    fm = _FM(nc, pool, h)
    P = GROUPS

    def seg(base, width):
        return stats[:, base : base + width]

    # ---- wide quantity APs (order (i, j, b) after permute) ----
    # Rxy(i,j,b): col = 49*(24+3b+j) + 3b+i = 1176 + 150b + 49j + i
    rxy = seg(1176, 1200).rearrange("p (b r) -> p b r", b=LANES)
    rxy = rxy[:, :, 0:147].rearrange("p b (j r2) -> p b j r2", j=3)[:, :, :, 0:3]
    rxy = rxy.rearrange("p b j i -> p i j b")
    # Sx(i,b): col = 2352 + 3b + i ; Sy(j,b): col = 2376 + 3b + j
    sxw = (
        seg(2352, 24)
        .rearrange("p (b i) -> p b i", b=LANES)
        .broadcast_to([P, LANES, 3, 3])
        .rearrange("p b i j -> p i j b")
    )
    syw = (
        seg(2376, 24)
        .rearrange("p (b j) -> p b j", b=LANES)
        .broadcast_to([P, LANES, 3, 3])
        .rearrange("p b j i -> p i j b")
    )

    n_ap = meta_ap
    rn = fm.recip(n_ap)
    rnw = rn.broadcast_to([P, LANES, 3, 3]).rearrange("p b i j -> p i j b")

    def w3(t):
        return t.rearrange("p (i j b) -> p i j b", i=3, j=3)

    # ---- C = Rxy - Sx Sy / n  (wide [P, 72], layout (i, j, b)) ----
    sxsy = fm.t(72)
    nc.vector.tensor_tensor(w3(sxsy[:]), sxw, syw, OP.mult)
    corr = fm.t(72)
    nc.vector.tensor_tensor(w3(corr[:]), w3(sxsy[:]), rnw, OP.mult)
    Ct = fm.t(72)
    nc.vector.tensor_tensor(w3(Ct[:]), rxy, w3(corr[:]), OP.subtract)

    def C(i, j):
        return Ct[:, (3 * i + j) * LANES : (3 * i + j + 1) * LANES]

    # ---- gx, gy packed [P, 16] = (x half | y half) per lane ----
    # Gxx_ii: col = 150b + 50i ; Gyy_ii: col = 1200 + 150b + 50i
    def diag(i):
        a = stats[:, 50 * i : 50 * i + 1200 + 150 * (LANES - 1) + 1 : 150]
        # cols 50i + 150k for k=0..15: k<8 -> Gxx lane k; k>=8 -> Gyy lane k-8
        return a

    gr = fm.add(fm.add(diag(0), diag(1)), diag(2))  # [P, 16]
    # |S|^2 packed: Sx cols 2352+3b+i, Sy cols 2376+3b+j -> one [P,48] square
    s48 = seg(2352, 48)
    sq = fm.mul(s48, s48)  # [P, 48]

    def sqs(i):
        return sq[:, i : i + 3 * 15 + 1 : 3]  # [P, 16]

    s2 = fm.add(fm.add(sqs(0), sqs(1)), sqs(2))  # [P, 16] (|Sx|^2 | |Sy|^2)
    rn16 = rn.broadcast_to([P, LANES, 2]).rearrange("p b h -> p h b")
    s2rn = fm.t(16)
    nc.vector.tensor_tensor(
        s2rn[:].rearrange("p (h b) -> p h b", h=2),
        s2.rearrange("p (h b) -> p h b", h=2),
        rn16,
        OP.mult,
    )
    g16 = fm.sub(gr, s2rn[:])  # [P, 16] = (gx | gy)
    gxy = fm.add(g16[:, 0:LANES], g16[:, LANES : 2 * LANES])  # gx + gy

    # ---- K = C^T C via one wide product + fold over i ----
    c4 = Ct[:].rearrange("p (i a b) -> p i a b", i=3, a=3)
    in0 = c4.broadcast_to([P, 3, 3, LANES, 3]).rearrange("p i a b bb -> p a bb i b")
    in1 = c4.broadcast_to([P, 3, 3, LANES, 3]).rearrange("p i bb b a -> p a bb i b")
    P3 = fm.t(216)
    nc.vector.tensor_tensor(
        P3[:].rearrange("p (a bb i b) -> p a bb i b", a=3, bb=3, i=3), in0, in1, OP.mult
    )
    p3v = P3[:].rearrange("p (a bb i b) -> p a bb i b", a=3, bb=3, i=3)
    # kkt is [P, 96] so the 32-strided diagonal view stays in-bounds; only
    # the first 72 columns hold K (layout (a, bb, b)).
    kkt = fm.t(96)
    kk01 = fm.t(72)
    nc.vector.tensor_tensor(
        kk01[:].rearrange("p (a bb b) -> p a bb b", a=3, bb=3),
        p3v[:, :, :, 0, :],
        p3v[:, :, :, 1, :],
        OP.add,
    )
    nc.vector.tensor_tensor(
        kkt[:, 0:72].rearrange("p (a bb b) -> p a bb b", a=3, bb=3),
        kk01[:].rearrange("p (a bb b) -> p a bb b", a=3, bb=3),
        p3v[:, :, :, 2, :],
        OP.add,
    )

    def kk(a, b):
        return kkt[:, (3 * a + b) * LANES : (3 * a + b + 1) * LANES]

    # ---- det(C) sign ----
    m0 = fm.sub(fm.mul(C(1, 1), C(2, 2)), fm.mul(C(1, 2), C(2, 1)))
    m1 = fm.sub(fm.mul(C(1, 0), C(2, 2)), fm.mul(C(1, 2), C(2, 0)))
    m2 = fm.sub(fm.mul(C(1, 0), C(2, 1)), fm.mul(C(1, 1), C(2, 0)))
    detC = fm.add(fm.sub(fm.mul(C(0, 0), m0), fm.mul(C(0, 1), m1)), fm.mul(C(0, 2), m2))

    # ---- eigen setup: q, p ----
    q = fm.ts(fm.add(fm.add(kk(0, 0), kk(1, 1)), kk(2, 2)), 1.0 / 3.0, OP.mult)
    kdiag = kkt[:, 0:96].rearrange("p (a r) -> p a r", a=3)[:, :, 0:LANES]  # [P,3,8]
    kdt = fm.t(24)
    nc.vector.tensor_tensor(
        kdt[:].rearrange("p (a b) -> p a b", a=3),
        kdiag,
        q.broadcast_to([P, LANES, 3]).rearrange("p b a -> p a b"),
        OP.subtract,
    )
    kd2 = fm.mul(kdt[:], kdt[:])  # [P, 24]
    p2 = fm.add(
        fm.add(kd2[:, 0:LANES], kd2[:, LANES : 2 * LANES]), kd2[:, 2 * LANES : 3 * LANES]
    )
    xsq = fm.add(
        fm.add(fm.mul(kk(0, 1), kk(0, 1)), fm.mul(kk(0, 2), kk(0, 2))),
        fm.mul(kk(1, 2), kk(1, 2)),
    )
    p2 = fm.stt(xsq, 2.0, p2, OP.mult, OP.add)
    p2c = fm.ts(fm.ts(p2, 1.0 / 6.0, OP.mult), 1e-30, OP.max)
    p = fm.sqrt(p2c)

    # ---- det(K - qI), r clamped ----
    kd = [kdt[:, a * LANES : (a + 1) * LANES] for a in range(3)]
    k01, k02, k12 = kk(0, 1), kk(0, 2), kk(1, 2)
    d0 = fm.mul(kd[0], fm.sub(fm.mul(kd[1], kd[2]), fm.mul(k12, k12)))
    d1 = fm.mul(k01, fm.sub(fm.mul(k01, kd[2]), fm.mul(k12, k02)))
    d2 = fm.mul(k02, fm.sub(fm.mul(k01, k12), fm.mul(kd[1], k02)))
    detKq = fm.add(fm.sub(d0, d1), d2)
    rp = fm.recip(p)
    r = fm.mul(fm.stt(detKq, 0.5, rp, OP.mult, OP.mult), fm.mul(rp, rp))
    r = fm.ts(fm.ts(r, 1.0, OP.min), -1.0, OP.max)

    # ---- acos(r)/3 via |r| fold + polynomial atan ----
    rabs = fm.stt(r, -1.0, r, OP.mult, OP.max)
    onemr = fm.ts2(rabs, -1.0, 1.0, OP.mult, OP.add)
    onepr = fm.ts(rabs, 1.0, OP.add)
    u = fm.mul(onemr, fm.recip(onepr))
    su = fm.sqrt(u)  # t = sqrt(u), A = 2 atan(t)
    at = fm.mul(su, fm.poly_u(u, ATAN_C))  # atan(t) = t * P(t^2)
    # acr = 2 at (1 - 2 rneg) + pi rneg ; phi = acr / 3
    rneg = fm.ts(r, 0.0, OP.is_lt)
    sgn = fm.ts2(rneg, -2.0, 1.0, OP.mult, OP.add)
    t1 = fm.mul(at, sgn)
    phi = fm.stt(rneg, float(np.pi / 3.0), fm.ts(t1, 2.0 / 3.0, OP.mult), OP.mult, OP.add)
    # c1 = cos(phi); c3m = cos(pi/3 - phi): pack both args, one poly chain
    w = fm.t(2 * LANES)
    nc.vector.tensor_copy(w[:, 0:LANES], phi)
    nc.vector.tensor_scalar(
        w[:, LANES : 2 * LANES], phi, -1.0, float(np.pi / 3.0), OP.mult, OP.add
    )
    wu = fm.tt(w[:], w[:], OP.mult)  # [P, 16]
    cw = fm.poly_u(wu, COS_C)
    c1 = cw[:, 0:LANES]
    c3m = cw[:, LANES : 2 * LANES]

    # ---- eigenvalues, packed sqrt ----
    p2x = fm.ts(p, 2.0, OP.mult)
    lt = fm.t(3 * LANES)
    nc.vector.tensor_tensor(lt[:, 0:LANES], q, fm.mul(p2x, c1), OP.add)  # l1
    nc.vector.tensor_tensor(
        lt[:, 2 * LANES : 3 * LANES], q, fm.mul(p2x, c3m), OP.subtract
    )  # l3
    nc.vector.tensor_tensor(
        lt[:, LANES : 2 * LANES],
        fm.stt(q, 3.0, lt[:, 0:LANES], OP.mult, OP.subtract),
        lt[:, 2 * LANES : 3 * LANES],
        OP.subtract,
    )  # l2 = 3q - l1 - l3
    ltc = fm.ts(lt[:], 0.0, OP.max)  # [P, 24] clamp
    s = fm.sqrt(ltc)

    # ---- trace with Kabsch sign, rmsd ----
    neg = fm.ts(detC, 0.0, OP.is_lt)
    d = fm.ts2(neg, -2.0, 1.0, OP.mult, OP.add)
    tr = fm.add(
        fm.add(s[:, 0:LANES], s[:, LANES : 2 * LANES]),
        fm.mul(d, s[:, 2 * LANES : 3 * LANES]),
    )
    diff = fm.stt(tr, -2.0, gxy, OP.mult, OP.add)
    msd = fm.mul(diff, rn)
    rmsd = fm.sqrt(fm.ts(msd, 0.0, OP.max))
    nc.vector.tensor_copy(out_ap, rmsd)


# ---------------------------------------------------------------------------
# Program builder. chunks: per-group chunk counts (len 64, same on all cores).
# ---------------------------------------------------------------------------
def build_program(chunks):
    chunks = list(chunks)
    assert len(chunks) == GROUPS
    colstart = np.concatenate([[0], np.cumsum(np.asarray(chunks) * ZCOLS)]).astype(int)
    TC = int(colstart[-1])

    install_tile_patch()
    nc = bass.Bass()
    z_d = nc.dram_tensor("z", [CHUNK, TC], BF16, kind="ExternalInput")
    meta_d = nc.dram_tensor("meta", [GROUPS, LANES], F32, kind="ExternalInput")
    out_d = nc.dram_tensor("out", [GROUPS, LANES], F32, kind="ExternalOutput")

    ZT = 16  # input DMA granularity: 4 groups per tile
    GPT = GROUPS // ZT

    with TileContext(nc) as tc:
        with (
            tc.tile_pool(name="const", bufs=1) as constp,
            tc.tile_pool(name="z", bufs=1) as zp,
            tc.tile_pool(name="pcopy", bufs=1) as pcp,
            tc.tile_pool(name="stats", bufs=1) as statp,
            tc.tile_pool(name="psum1", bufs=2, space="PSUM") as ps1p,
            tc.tile_pool(name="psum2", bufs=1, space="PSUM") as ps2p,
        ):
            # Input tiles first: start the stream as early as possible.
            zt = []
            for t in range(ZT):
                c0 = int(colstart[t * GPT])
                c1 = int(colstart[(t + 1) * GPT])
                tile = zp.tile([CHUNK, c1 - c0], BF16, tag=f"z{t}", name=f"z{t}")
                nc.sync.dma_start(out=tile[:], in_=z_d[:, c0:c1])
                zt.append((tile, c0))

            meta_t = constp.tile([GROUPS, LANES], F32)
            nc.sync.dma_start(out=meta_t[:], in_=meta_d[:])
            ident = constp.tile([ZCOLS, ZCOLS], F32)
            make_identity(nc, ident[:])
            # Pre-load the Sqrt activation table (the only table we use).
            scr = constp.tile([GROUPS, 1], F32)
            nc.vector.memset(scr[:], 1.0)
            nc.scalar.activation(scr[:], scr[:], AF.Sqrt)

            P_all = pcp.tile([ZCOLS, GROUPS * ZCOLS], F32, tag="Pall")
            # psum2: 5 bank-sized tiles, 10 (last 9) Gram columns each.
            ps2 = [
                ps2p.tile(
                    [GROUPS, (10 if t < 4 else 9) * ZCOLS],
                    F32,
                    tag=f"ps2{t}",
                    name=f"ps2{t}",
                )
                for t in range(5)
            ]
            stats = statp.tile([GROUPS, PP], F32)
            out_t = statp.tile([GROUPS, LANES], F32)

            def emit_block(j):
                p1 = ps1p.tile([ZCOLS, BLK * ZCOLS], F32, tag="p1", name=f"p1_{j}")
                for g in range(BLK):
                    q = j * BLK + g
                    tile, c0 = zt[q // GPT]
                    tgt = p1[:, ZCOLS * g : ZCOLS * (g + 1)]
                    nchunks = chunks[q]
                    for k in range(nchunks):
                        o = int(colstart[q]) - c0 + k * ZCOLS
                        zs = tile[:, o : o + ZCOLS]
                        nc.tensor.matmul(
                            tgt, zs, zs, start=(k == 0), stop=(k == nchunks - 1)
                        )
                o = j * BLK * ZCOLS
                nc.scalar.activation(
                    P_all[:, o : o + BLK * ZCOLS], p1[:], AF.Identity
                )

            def emit_extract():
                # Per Gram column c, one transpose [49, 64] -> [64, 49] into
                # a psum2 bank tile, then PSUM -> stats SBUF (ScalarE + DVE).
                for c in range(ZCOLS):
                    src = P_all[:, c : GROUPS * ZCOLS : ZCOLS]
                    t = c // 10
                    dst = ps2[t][:, ZCOLS * (c - 10 * t) : ZCOLS * (c - 10 * t + 1)]
                    nc.tensor.transpose(dst, src, ident[:])
                for t in range(5):
                    w = (10 if t < 4 else 9) * ZCOLS
                    dst = stats[:, 490 * t : 490 * t + w]
                    if t < 3:
                        nc.scalar.activation(dst, ps2[t][:], AF.Identity)
                    else:
                        nc.vector.tensor_copy(dst, ps2[t][:])

            for j in range(NBLK):
                emit_block(j)
            emit_extract()
            _emit_final_pass(nc, statp, 0, stats[:], meta_t[:], out_t[:])

            nc.sync.dma_start(out=out_d[:], in_=out_t[:])

    return nc


# ---------------------------------------------------------------------------
# Host side
# ---------------------------------------------------------------------------
def plan_shards(num_atoms):
    na = np.asarray(num_atoms).astype(np.int64)
    B = na.shape[0]
    assert B == GROUPS * GROUP_ROWS, f"unsupported batch {B}"
    order = np.argsort(na, kind="stable")
    caps = na[order].reshape(GROUPS, GROUP_ROWS)[:, -1]
    chunks = np.maximum(1, -(-caps // CHUNK)).astype(int)  # ceil
    return order, chunks


def shard_inputs(coords_input, coords_target, num_atoms, order, chunks):
    B, f = coords_input.shape
    nmax = f // 3
    na = np.asarray(num_atoms).astype(np.int64)
    x3 = coords_input.reshape(B, nmax, 3)
    y3 = coords_target.reshape(B, nmax, 3)
    colstart = np.concatenate([[0], np.cumsum(chunks * ZCOLS)]).astype(int)
    TC = int(colstart[-1])

    in_maps = []
    for c in range(N_CORES):
        z = np.zeros((CHUNK, TC), dtype=ml_dtypes.bfloat16)
        meta = np.empty((GROUPS, LANES), np.float32)
        for v in np.unique(chunks):
            qs = np.where(chunks == v)[0]
            nq = len(qs)
            A = int(v) * CHUNK
            # rows for (q, b): order[q*64 + b*8 + c]
            ridx = order[
                (qs[:, None] * GROUP_ROWS) + np.arange(LANES)[None, :] * N_CORES + c
            ]
            nar = na[ridx]  # [nq, LANES]
            meta[qs, :] = nar.astype(np.float32)
            mask = (np.arange(A)[None, None, :] < nar[:, :, None]).astype(np.float32)
            xa = x3[ridx.ravel(), :A, :].reshape(nq, LANES, A, 3) * mask[..., None]
            ya = y3[ridx.ravel(), :A, :].reshape(nq, LANES, A, 3) * mask[..., None]
            xt = xa.reshape(nq, LANES, int(v), CHUNK, 3).transpose(0, 2, 3, 1, 4)
            yt = ya.reshape(nq, LANES, int(v), CHUNK, 3).transpose(0, 2, 3, 1, 4)
            buf = np.empty((nq, int(v), CHUNK, ZCOLS), np.float32)
            buf[..., 0 : 3 * LANES] = xt.reshape(nq, int(v), CHUNK, 3 * LANES)
            buf[..., 3 * LANES : 6 * LANES] = yt.reshape(nq, int(v), CHUNK, 3 * LANES)
            buf[..., 6 * LANES] = 1.0
            colidx = (
                colstart[qs][:, None] + np.arange(int(v) * ZCOLS)[None, :]
            ).ravel()
            z[:, colidx] = (
                buf.transpose(2, 0, 1, 3).reshape(CHUNK, nq * int(v) * ZCOLS)
            ).astype(ml_dtypes.bfloat16)
        in_maps.append({"z": z, "meta": meta})
    return in_maps


def unshard_outputs(results, order, B):
    out = np.empty(B, dtype=np.float32)
    for c in range(N_CORES):
        o = np.asarray(results[c]["out"], np.float32)  # [GROUPS, LANES]
        q = np.arange(GROUPS)[:, None]
        b = np.arange(LANES)[None, :]
        rows = order[q * GROUP_ROWS + b * N_CORES + c]
        out[rows] = o
    return out


# ---------------------------------------------------------------------------
# Entry point: full inputs in, full output out. Shards across 8 NeuronCores.
# ---------------------------------------------------------------------------
_PROG_CACHE = {}


def _get_program(chunks):
    key = tuple(int(v) for v in chunks)
    if key not in _PROG_CACHE:
        _PROG_CACHE[key] = build_program(list(key))
    return _PROG_CACHE[key]


def kernel(coords_input, coords_target, num_atoms):
    from concourse.bass_utils import run_bass_kernel_spmd

    x = np.ascontiguousarray(np.asarray(coords_input, dtype=np.float32))
    y = np.ascontiguousarray(np.asarray(coords_target, dtype=np.float32))
    na = np.asarray(num_atoms).astype(np.int64)
    B = x.shape[0]

    order, chunks = plan_shards(na)
    in_maps = shard_inputs(x, y, na, order, chunks)
    nc = _get_program(chunks)
    res = run_bass_kernel_spmd(nc, in_maps, core_ids=list(range(N_CORES)))
    return unshard_outputs(res.results, order, B).astype(np.float32)
